# revision 11
# baseline (speedup 1.0000x reference)
"""Trainium2 Bass kernel for DecoupledSOLOHead mask decoding + Matrix NMS.

Math (reference):
    mask_x = seg_preds_x[x_inds]; mask_y = seg_preds_y[y_inds]   # [N,H,W]
    soft = mask_x*mask_y; hard = soft > THR
    sum_masks = hard.sum((1,2)); seg_score = (soft*hard).sum((1,2))/max(sm,1)
    scores = cate_scores * seg_score
    inter = hard_flat @ hard_flat.T          # [N,N]
    ... matrix NMS (gaussian) -> scores * decay_coef

Strategy (8 cores), v2:
  - Shard the H*W=60800 pixel dim: 7600 px/core, zero-padded to 7680 = 30
    superchunks of 256 pixels (2 halves of 128).
  - LOG-SPACE gather: host ships lx=max(log x,-30), ly=max(log y,-30) as
    bf16 slabs.  Per 128-px half, ONE PSUM accumulates
    s = lx^T@ohx + ly^T@ohy (2 bf16 matmuls) so soft>THR becomes
    s > log(THR); no fp32 hi/lo split, no PSUM-bounce copy, no DVE mult.
  - hard = (s > lnTHR) in fp8e4 (DVE); exps = exp(s) in fp8e4 (Scalar).
  - inter via fp8 DoubleRow matmuls (2 k-tiles = the 2 halves; binary
    inputs -> exact integer PSUM).  The NMS mask (same-label & i<j) makes
    S strictly lower-triangular in use, so tile m only needs columns
    < 125(m+1): inter work, AllReduce payload, readback and decay all
    shrink by ~37%.
  - num = sum(exp(s)) over ALL pixels (sub-threshold tail adds +3e-4 rel
    bias, well within tolerance); sm = sum(hard).  Both are fp8-DR
    matmuls against a ones vector -> no diag extraction epilogue.
  - One u16 AllReduce of [S_tri | num | sm] (~315 KB), staged over two HW
    DMA queues (sync + scalar engines).
  - Decay stage replicated on every core, engine-split across DVE / Pool
    / Scalar, on truncated widths.  dec = min(1, min over masked ratio)
    is exact because some column always has comp=0 (e.g. column 0).
    dm = exp(q), q = -sigma*mask*iou^2 folds the mask via a host tensor
    maskS = -sigma*mask; comp term exp(+sigma*comp^2) = exp(-min q).
"""

import sys

if "/opt/trn_rl_repo" not in sys.path:
    sys.path.insert(0, "/opt/trn_rl_repo")

from contextlib import ExitStack

import numpy as np
import ml_dtypes

import bass_rust
import concourse.bass as bass
import concourse.tile as tile
from concourse import bacc, mybir
from concourse.bass_utils import run_bass_kernel_spmd

N = 500
G = 128
H, W = 200, 304
HW = H * W              # 60800
NCORES = 8
PPC = HW // NCORES      # 7600 pixels per core
PAD = 7680              # padded to 30 superchunks of 256
SCH = PAD // 256        # 30
MT = 125                # candidate tile (4 tiles of 125 = 500)
THR = 0.005
LNTHR = float(np.log(THR))
SIGMA = 2.0

BF16 = mybir.dt.bfloat16
FP8 = mybir.dt.float8e4
F32 = mybir.dt.float32
U16 = mybir.dt.uint16
ALU = mybir.AluOpType
AFT = bass_rust.ActivationFunctionType
DR = mybir.MatmulPerfMode.DoubleRow

# truncated tile widths and cc buffer layout (flat u16)
TW = [MT * (m + 1) for m in range(4)]          # 125,250,375,500
SOFF = [0]
for m in range(4):
    SOFF.append(SOFF[-1] + MT * TW[m])
CC_NUM = SOFF[4]                # 156250
CC_SM = CC_NUM + N              # 156750
CC_LEN = CC_NUM + 2 * N         # 157250

_NC_CACHE = []


def _r2(ap, f):
    """reshape a flat (1-D) AP slice to [p, f]"""
    return ap.rearrange("(p f) -> p f", f=f)


def _bcast(ap_flat, p, n):
    """partition-broadcast AP: read the same n elements into p partitions"""
    return bass.AP(tensor=ap_flat.tensor, offset=ap_flat.offset,
                   ap=[[0, p], [1, n]])


def _build_nc():
    nc = bacc.Bacc("TRN2", target_bir_lowering=False, debug=False,
                   num_devices=NCORES)

    lx_d = nc.dram_tensor("lx", [G, PAD], BF16, kind="ExternalInput")
    ly_d = nc.dram_tensor("ly", [G, PAD], BF16, kind="ExternalInput")
    ohx_d = nc.dram_tensor("ohx", [G, N], BF16, kind="ExternalInput")
    ohy_d = nc.dram_tensor("ohy", [G, N], BF16, kind="ExternalInput")
    # maskS_cat[j, o_t+i] = -SIGMA if (labels[i]==labels[125t+j] and
    #                        i < 125t+j) else 0, tiles concatenated on the
    #                        free dim at offsets o=[0,125,375,750], w=TW
    maskS_d = nc.dram_tensor("maskS", [MT, 1250], BF16, kind="ExternalInput")
    cate_d = nc.dram_tensor("cate", [1, N], F32, kind="ExternalInput")
    out_d = nc.dram_tensor("out", [1, N], F32, kind="ExternalOutput")

    with tile.TileContext(nc) as tc, ExitStack() as ctx:
        consts = ctx.enter_context(tc.tile_pool(name="consts", bufs=1))
        work = ctx.enter_context(tc.tile_pool(name="work", bufs=3))
        fin = ctx.enter_context(tc.tile_pool(name="fin", bufs=1))
        psS = ctx.enter_context(tc.tile_pool(name="psS", bufs=1, space="PSUM"))
        psG = ctx.enter_context(tc.tile_pool(name="psG", bufs=1, space="PSUM"))
        dram = ctx.enter_context(tc.tile_pool(name="dram", bufs=1, space="DRAM"))

        # ---- tiny tensors first so chunk 0 can start ASAP ----
        ohx_s = consts.tile([G, N], BF16)
        nc.sync.dma_start(ohx_s[:], ohx_d[:])
        ohy_s = consts.tile([G, N], BF16)
        nc.sync.dma_start(ohy_s[:], ohy_d[:])
        cate_s = consts.tile([1, N], F32)
        nc.scalar.dma_start(cate_s[:], cate_d[:])
        maskS_s = consts.tile([MT, 1250], BF16)
        nc.scalar.dma_start(maskS_s[:], maskS_d[:])
        # dual-fp8 LDWEIGHTS needs the k-tile-pair stride 16B-aligned
        ones2 = consts.tile([G, 32], FP8)
        nc.vector.memset(ones2[:], 1.0)

        # ---- slabs, piece-major, split across the two HW DMA queues ----
        lx_s = consts.tile([G, PAD], BF16)
        ly_s = consts.tile([G, PAD], BF16)
        NP = 10
        PW = PAD // NP
        for p in range(NP):
            sl = np.s_[:, p * PW:(p + 1) * PW]
            nc.sync.dma_start(lx_s[sl], lx_d[sl])
            nc.scalar.dma_start(ly_s[sl], ly_d[sl])

        # ---- PSUM: tiles 0-2 [125,w], tile 3 [126,500] (row 125 = sm via
        # a ones pad column in hard), num row = 5 banks; s_ps bufs=2 ----
        s_ps = [psS.tile([MT + (1 if m == 3 else 0), TW[m]], F32,
                         name=f"s_ps{m}") for m in range(4)]
        num_ps = psS.tile([1, N], F32)

        # ---- superchunk loop ----
        for c in range(SCH):
            first, last = (c == 0), (c == SCH - 1)
            hard = work.tile([G, 1024], FP8, tag="hard", name="hard")
            exps = work.tile([G, 1024], FP8, tag="exps", name="exps")
            # ones in the pad column after each half's 500 candidates: the
            # m=3 lhsT is widened to 126 so PSUM row 125 accumulates sm
            nc.gpsimd.memset(hard[:, 500:501], 1.0)
            nc.gpsimd.memset(hard[:, 1012:1013], 1.0)
            for h in range(2):
                cs = np.s_[:, (2 * c + h) * 128:(2 * c + h + 1) * 128]
                sps = psG.tile([128, N], F32, tag="sps", bufs=2, name="sps")
                nc.tensor.matmul(sps[:], lx_s[cs], ohx_s[:], start=True,
                                 stop=False)
                nc.tensor.matmul(sps[:], ly_s[cs], ohy_s[:], start=False,
                                 stop=True)
                hs = np.s_[:, h * 512:h * 512 + N]
                nc.vector.tensor_scalar(hard[hs], sps[:], LNTHR, None,
                                        op0=ALU.is_gt)
                nc.scalar.activation(exps[hs], sps[:], AFT.Exp)

            hard2 = hard[:].rearrange("p (two f) -> p two f", two=2)
            exps2 = exps[:].rearrange("p (two f) -> p two f", two=2)
            ones2r = ones2[:, :32].rearrange("p (two f) -> p two f", two=2)[:, :, :1]
            for m in range(4):
                mw = MT + (1 if m == 3 else 0)
                nc.tensor.matmul(s_ps[m][:],
                                 hard2[:, :, MT * m:MT * m + mw],
                                 hard2[:, :, :TW[m]],
                                 start=first, stop=last, perf_mode=DR)
            nc.tensor.matmul(num_ps[:], ones2r, exps2[:, :, :N], start=first,
                             stop=last, perf_mode=DR)

        # ---- epilogue: convert to u16, stage into cc buffer ----
        cc_in = dram.tile([CC_LEN], U16)
        cc_out = dram.tile([CC_LEN], U16, addr_space="Shared")
        for m in range(4):
            mw = MT + (1 if m == 3 else 0)
            s16 = fin.tile([mw, TW[m]], U16, name=f"s16_{m}")
            if m % 2 == 0:
                nc.vector.tensor_copy(s16[:], s_ps[m][:])
            else:
                nc.scalar.copy(s16[:], s_ps[m][:])
            q = nc.sync if m % 2 == 0 else nc.scalar
            q.dma_start(_r2(cc_in[SOFF[m]:SOFF[m + 1]], TW[m]), s16[:MT, :])
            if m == 3:   # row 125 = sm (exact integers, trunc-safe)
                nc.scalar.dma_start(_r2(cc_in[CC_SM:CC_SM + N], N),
                                    s16[MT:MT + 1, :])
        # num: +0.5 so trunc-style conversion rounds to nearest
        numr_f = fin.tile([1, N], F32)
        nc.vector.tensor_scalar(numr_f[:], num_ps[:], 0.5, None, op0=ALU.add)
        num16 = fin.tile([1, N], U16)
        nc.vector.tensor_copy(num16[:], numr_f[:])
        nc.sync.dma_start(_r2(cc_in[CC_NUM:CC_NUM + N], N), num16[:])

        # ---- u16 AllReduce of [S_tri | num | sm] ----
        nc.gpsimd.collective_compute(
            "AllReduce", ALU.add, replica_groups=[list(range(NCORES))],
            ins=[cc_in.opt()], outs=[cc_out.opt()])

        # ---- readback (small rows first; S tiles into one concatenated
        # [125, 1250] layout, split over 2 queues) ----
        CO = [0, 125, 375, 750]  # free-dim offsets of the 4 tiles
        numr = fin.tile([1, N], U16)
        nc.sync.dma_start(numr[:], _r2(cc_out[CC_NUM:CC_NUM + N], N))
        smr = fin.tile([1, N], U16)
        nc.sync.dma_start(smr[:], _r2(cc_out[CC_SM:CC_SM + N], N))
        st = fin.tile([MT, 1250], U16)
        smb = fin.tile([MT, 1250], U16)   # sm[i] bcast down partitions
        smc = []
        for t in range(4):
            w = np.s_[:, CO[t]:CO[t] + TW[t]]
            q = nc.sync if t % 2 == 0 else nc.scalar
            q.dma_start(st[w], _r2(cc_out[SOFF[t]:SOFF[t + 1]], TW[t]))
            nc.gpsimd.dma_start(smb[w],
                                _bcast(cc_out[CC_SM:CC_SM + TW[t]], MT, TW[t]))
            s = fin.tile([MT, 1], U16, name=f"smc{t}")
            nc.scalar.dma_start(
                s[:], _r2(cc_out[CC_SM + MT * t:CC_SM + MT * (t + 1)], 1))
            smc.append(s)

        # scores row = cate * num / max(sm, 1)
        smx = fin.tile([1, N], F32)
        nc.vector.tensor_scalar(smx[:], smr[:], 1.0, None, op0=ALU.max)
        rs = fin.tile([1, N], F32)
        nc.vector.reciprocal_approx_fast(rs[:], smx[:])
        sc1 = fin.tile([1, N], F32)
        nc.vector.tensor_tensor(sc1[:], numr[:], rs[:], op=ALU.mult)
        scores = fin.tile([1, N], F32)
        nc.vector.tensor_tensor(scores[:], sc1[:], cate_s[:], op=ALU.mult)

        scr_a = dram.tile([N], F32)   # rcomp bounce (column -> row)
        scr_b = dram.tile([N], F32)   # decay bounce
        # decay, all 4 tiles concatenated [125, 1250]:
        #   u = (sm_i + sm_j) - S;  iou = S / u  (>=0)
        #   q = maskS * iou^2  (maskS = -SIGMA*mask; q<=0)
        #   dm = exp(q);  rcomp[j] = exp(-min_i q)  [= exp(+SIGMA*comp^2)]
        #   dec[j] = min(1, min_i dm*rcomp_bcast)
        u = work.tile([MT, 1250], F32, tag="dk", name="u")
        for t in range(4):
            w = np.s_[:, CO[t]:CO[t] + TW[t]]
            nc.vector.scalar_tensor_tensor(u[w], smb[w], smc[t][:], st[w],
                                           op0=ALU.add, op1=ALU.subtract)
        ru = work.tile([MT, 1250], F32, tag="dk", name="ru")
        nc.vector.reciprocal_approx_fast(ru[:], u[:])
        iou = work.tile([MT, 1250], F32, tag="dk", name="iou")
        nc.vector.tensor_tensor(iou[:], st[:], ru[:], op=ALU.mult)
        sq = work.tile([MT, 1250], F32, tag="dk", name="sq")
        nc.scalar.activation(sq[:], iou[:], AFT.Square)
        qc = fin.tile([MT, 1250], F32, name="qc")
        nc.vector.tensor_tensor(qc[:], sq[:], maskS_s[:], op=ALU.mult)
        qp = fin.tile([MT, 4], F32, name="qp")
        for t in range(4):
            nc.vector.tensor_reduce(qp[:, t:t + 1],
                                    qc[:, CO[t]:CO[t] + TW[t]],
                                    axis=mybir.AxisListType.X, op=ALU.min)
        rcm = fin.tile([MT, 4], F32, name="rcm")
        nc.scalar.activation(rcm[:], qp[:], AFT.Exp, scale=-1.0)
        for t in range(4):
            nc.sync.dma_start(_r2(scr_a[MT * t:MT * (t + 1)], 1),
                              rcm[:, t:t + 1])
        dm = work.tile([MT, 1250], F32, tag="dk", name="dm")
        nc.scalar.activation(dm[:], qc[:], AFT.Exp)
        rcb = fin.tile([MT, 1250], F32)
        for t in range(4):
            nc.gpsimd.dma_start(rcb[:, CO[t]:CO[t] + TW[t]],
                                _bcast(scr_a[:TW[t]], MT, TW[t]))
        ratio = work.tile([MT, 1250], F32, tag="dk", name="ratio")
        nc.vector.tensor_tensor(ratio[:], dm[:], rcb[:], op=ALU.mult)
        dp = fin.tile([MT, 4], F32, name="dp")
        for t in range(4):
            nc.vector.tensor_reduce(dp[:, t:t + 1],
                                    ratio[:, CO[t]:CO[t] + TW[t]],
                                    axis=mybir.AxisListType.X, op=ALU.min)
        dec = fin.tile([MT, 4], F32, name="dec")
        nc.vector.tensor_scalar(dec[:], dp[:], 1.0, None, op0=ALU.min)
        for t in range(4):
            nc.scalar.dma_start(_r2(scr_b[MT * t:MT * (t + 1)], 1),
                                dec[:, t:t + 1])
        decrow = fin.tile([1, N], F32)
        nc.sync.dma_start(decrow[:], _r2(scr_b[:], N))
        res = fin.tile([1, N], F32)
        nc.vector.tensor_tensor(res[:], scores[:], decrow[:], op=ALU.mult)
        nc.sync.dma_start(out_d[:], res[:])

    nc.compile()
    return nc


def _get_nc():
    if not _NC_CACHE:
        _NC_CACHE.append(_build_nc())
    return _NC_CACHE[0]


def _prep_inputs(cate_scores, seg_preds_x, seg_preds_y, cate_labels, x_inds,
                 y_inds):
    bf16 = ml_dtypes.bfloat16
    X = np.asarray(seg_preds_x, np.float32).reshape(G, HW)
    Y = np.asarray(seg_preds_y, np.float32).reshape(G, HW)
    with np.errstate(divide="ignore"):
        lx = np.maximum(np.log(X), -30.0).astype(bf16)
        ly = np.maximum(np.log(Y), -30.0).astype(bf16)

    xi = np.asarray(x_inds).astype(np.int64)
    yi = np.asarray(y_inds).astype(np.int64)
    lab = np.asarray(cate_labels).astype(np.int64)
    ohx = (np.arange(G)[:, None] == xi[None, :]).astype(bf16)
    ohy = (np.arange(G)[:, None] == yi[None, :]).astype(bf16)

    jj = np.arange(N)
    maskF = (-SIGMA * ((lab[None, :] == lab[:, None]) &
                       (jj[None, :] < jj[:, None]))).astype(bf16)
    CO = [0, 125, 375, 750]
    maskS = np.zeros((MT, 1250), bf16)
    for t in range(4):
        maskS[:, CO[t]:CO[t] + MT * (t + 1)] = \
            maskF[MT * t:MT * (t + 1), :MT * (t + 1)]
    cate = np.asarray(cate_scores, np.float32).reshape(1, N)

    in_maps = []
    for k in range(NCORES):
        sl = np.s_[:, k * PPC:(k + 1) * PPC]
        m = {}
        for name, arr in (("lx", lx), ("ly", ly)):
            s = np.full((G, PAD), -30.0, bf16)
            s[:, :PPC] = arr[sl]
            m[name] = s
        m["ohx"] = ohx
        m["ohy"] = ohy
        m["maskS"] = maskS
        m["cate"] = cate
        in_maps.append(m)
    return in_maps


def kernel(**inputs) -> np.ndarray:
    in_maps = _prep_inputs(**inputs)
    nc = _get_nc()
    res = run_bass_kernel_spmd(nc, in_maps, core_ids=list(range(NCORES)))
    return np.asarray(res.results[0]["out"], np.float32).reshape(N)


if __name__ == "__main__":
    rng = np.random.default_rng(0)
    inputs = dict(
        cate_scores=rng.random(N, np.float32),
        seg_preds_x=rng.random((G, H, W), np.float32),
        seg_preds_y=rng.random((G, H, W), np.float32),
        cate_labels=rng.integers(0, 80, N),
        x_inds=rng.integers(0, G, N),
        y_inds=rng.integers(0, G, N),
    )
    out = kernel(**inputs)
    print(out[:10])
